# revision 51
# baseline (speedup 1.0000x reference)
"""Tensor-parallel GQA attention block on 8 TRN2 NeuronCores (Bass/Tile).

Problem: B=1, S=2048, DIM=4096, 32 q heads / 8 kv heads (GQA), head_dim=128,
RoPE, causal softmax, output projection.

Sharding (tensor parallel by head, per the hint): core c of 8 owns q heads
4c..4c+3 and kv head c (GQA groups stay with their q heads). wqkv rows and wo
columns are sharded by head; attention is fully local per core; each core
emits a partial (S, DIM) output (its heads through its wo column slice) and
the partials are summed on the host at unshard time (the "all-reduce after
wo" of the hint, done off-device since full I/O passes through the host
anyway).

Per-core device kernel -- all operands host-pre-transposed so every matmul has
its contraction dim on SBUF partitions; zero on-device transposes:
  qkT = wqkT.T @ xT              (head dims on partitions, seq free)
  v   = xT.T @ wvT               (seq on partitions, head dim free)
  RoPE on qT/kT in transposed layout: host permutes rows into re(0..63)/
    im(64..127); cos/sin arrive as stacked (128, S) tables [cos;cos] and
    [-sin;sin]; 1/sqrt(HD) is folded into wq on the host.
  per head, per 512-wide q chunk (causal: only k tiles <= chunk end):
    S.T[j] = kT_j.T @ qT_chunk   (k positions on partitions => softmax
                                  denominators via a ones-matmul; no P
                                  transpose anywhere)
    P.T[j] = exp(S.T[j] - 12)    (triangular mask added on diagonal tiles;
                                  N trimmed to the causal columns)
    sums  += ones128.T @ P.T[j]  (PSUM-accumulated, rows replicated)
    O.T   += matmul(lhsT=V_j, rhs=P.T[j])
    O.T_norm = O.T * reciprocal_approx(sums)  -> bf16
  out[t, d] = sum_h O.T_h[:, t].T @ woT_h[:, d]

Schedule: quantum-interleaved emission.  Each phase is a generator of PE
"quanta" (~0.6-0.9us of matmuls); a weighted-fair driver mixes them so that
during attention the exp-dependent ones/PV matmuls always have independent
projection/output quanta between them and their score matmul -- the ACT
engine's exp latency hides under PE work instead of stalling it.  Each B
phase's first j-steps are pre-started inside the previous C interleave
(shared generator) so their counting-semaphore thresholds exclude C's
final PSUM evictions.  Weights stream as 9 staggered group DMAs of a
combined wqkv tensor (SWDGE descriptor generation is ~0.7us per DMA, so
descriptor count, not just bytes, gates startup); wo and per-chunk cos/sin
slices load outside the startup window.  PSUM->SBUF evictions alternate
ACT/DVE and out-DMAs rotate across engine queues (4-way at the tail).

Compute in bf16 with f32 PSUM accumulation; rel l2 error vs the f32
reference is ~8e-3.
"""
import sys

sys.path.insert(0, "/opt/trn_rl_repo")

from contextlib import ExitStack

import numpy as np
import ml_dtypes

import concourse.bass as bass
import concourse.tile as tile
import concourse.mybir as mybir
from concourse import bacc
from concourse.bass_utils import run_bass_kernel_spmd

F32 = mybir.dt.float32
BF16 = mybir.dt.bfloat16
NPBF16 = ml_dtypes.bfloat16

NH, NKV, HD = 32, 8, 128
S, DIM = 2048, 4096
N_CORES = 8
NHL = NH // N_CORES          # q heads per core
PERM = np.concatenate([np.arange(0, 128, 2), np.arange(1, 128, 2)])
WGROUPS = [2, 2, 4, 4, 4, 4, 4, 4, 4]   # k-tiles per weight-group DMA


def build_attention_kernel(nc, S=2048, DIM=4096, C=12.0):
    NHL = 4          # local q heads
    HD = 128
    CHUNK = 512
    P = 128
    NKT = DIM // P         # k tiles over model dim
    NCH = S // CHUNK       # seq chunks
    QKM = NHL + 1          # m-tiles in qk GEMM (4 q heads + 1 k head)
    NDC = DIM // CHUNK     # output dim chunks
    WM = QKM * P + HD      # combined wqkv row width (640 qk + 128 v)

    # ---- DRAM I/O ----
    # x, host pre-swizzled into 32 contiguous 512KB sub-slabs (8 k-tiles x
    # one 256-col half-chunk each, partition-major): one DMA instruction per
    # sub-slab instead of one per (k, hc) tile -- the HWDGE queue's ~0.66us
    # per-instruction cost made per-tile x fetches the feeding bottleneck.
    x5 = nc.dram_tensor("x5", (2 * (S // 512) * 4, P, 8, 256), BF16,
                        kind="ExternalInput").ap()
    # combined qkv weights, host pre-swizzled to (p, kt, m): columns 0..639 =
    # wqkT (4 q heads + 1 k head), 640..767 = wvT -- each k-tile-group DMA is
    # a contiguous slab read (a strided (kt p)->p kt gather costs far more
    # SWDGE descriptor time and delays the first matmul).
    wqkvT = nc.dram_tensor("wqkvT", (P, NKT, WM), BF16,
                           kind="ExternalInput").ap()
    woT = nc.dram_tensor("woT", (P, NHL, DIM), BF16, kind="ExternalInput").ap()
    # csT[:, ch, 0, :] = cos columns of chunk ch ([cos;cos] stacked rows),
    # csT[:, ch, 1, :] = sin columns ([-sin;+sin]) -- one DMA per chunk.
    csT = nc.dram_tensor("csT", (128, NCH, 2, CHUNK), F32,
                         kind="ExternalInput").ap()
    onesW = nc.dram_tensor("onesW", (P, P), BF16, kind="ExternalInput").ap()
    maskT = nc.dram_tensor("maskT", (P, P), F32, kind="ExternalInput").ap()
    out = nc.dram_tensor("out", (S, DIM), BF16, kind="ExternalOutput").ap()

    with tile.TileContext(nc) as tc, ExitStack() as ctx:
        const = ctx.enter_context(tc.tile_pool(name="const", bufs=1))
        resid = ctx.enter_context(tc.tile_pool(name="resid", bufs=1))
        xpool = ctx.enter_context(tc.tile_pool(name="xp", bufs=5))
        ptpool = ctx.enter_context(tc.tile_pool(name="ptp", bufs=6))
        tmppool = ctx.enter_context(tc.tile_pool(name="tmp", bufs=4))
        obpool = ctx.enter_context(tc.tile_pool(name="obp", bufs=8))
        psum = ctx.enter_context(tc.tile_pool(name="psum", bufs=8, space="PSUM"))

        # ---- weights: 9 staggered group DMAs (small groups first so the
        # first matmul starts in ~2us, larger ones amortize the per-DMA
        # SWDGE descriptor cost while the wire streams). ----
        w_g = []
        k0 = 0
        for gi, gn in enumerate(WGROUPS):
            g = const.tile([P, gn, WM], BF16, tag=f"wg{gi}", name=f"wg{gi}")
            w_g.append((g, k0, gn, gi))
            k0 += gn

        def emit_wdma(lo, hi):
            for g, k0, gn, gi in w_g[lo:hi]:
                # alternate queues: the ~4.5us first-DMA latency and the
                # wire time are paid in parallel across two otherwise-idle
                # queues.
                eng = nc.scalar if gi % 2 == 0 else nc.gpsimd
                eng.dma_start(g[:], wqkvT[:, k0:k0 + gn, :])

        # only the first 4 groups (k-tiles 0..11) stream up front -- the
        # rest would steal wire bandwidth from the first x slabs; they are
        # emitted from inside gen_a_full once the pipeline is rolling.
        emit_wdma(0, 4)
        ktile = []
        for g, k0, gn, gi in w_g:
            for i in range(gn):
                ktile.append(g[:, i])
        wqk_sb = [ktile[k][:, 0:QKM * P] for k in range(NKT)]
        wv_sb = [ktile[k][:, QKM * P:WM] for k in range(NKT)]
        ones_sb = const.tile([P, P], BF16, tag="ones", name="ones")
        nc.gpsimd.dma_start(ones_sb[:], onesW[:])
        mask_sb = const.tile([P, P], F32, tag="mask", name="mask")
        nc.gpsimd.dma_start(mask_sb[:], maskT[:])
        negC = const.tile([P, 1], F32, tag="negC", name="negC")
        nc.any.memset(negC[:], -C)
        cs_sb = const.tile([P, NCH, 2, CHUNK], F32, tag="cs", name="cs")
        wo_sb = const.tile([P, NHL, DIM], BF16, tag="wo", name="wo")

        cs_loaded = [False] * NCH

        def load_cs(ch):
            """cos/sin columns for chunk ch -- emitted at the start of the
            chunk's first A half so the slice has a full phase to arrive."""
            if cs_loaded[ch]:
                return
            cs_loaded[ch] = True
            nc.gpsimd.dma_start(cs_sb[:, ch], csT[:, ch])

        load_cs(0)
        load_cs(1)

        def load_wo(half):
            sl = slice(half * (DIM // 2), (half + 1) * (DIM // 2))
            nc.gpsimd.dma_start(wo_sb[:, :, sl], woT[:, :, sl])

        # resident activations (per chunk tiles for fine-grained deps)
        q_sb = [[resid.tile([P, CHUNK], BF16, tag=f"q{h}_{ch}", name=f"q{h}_{ch}")
                 for ch in range(NCH)] for h in range(NHL)]
        k_sb = [resid.tile([P, CHUNK], BF16, tag=f"k{ch}", name=f"k{ch}")
                for ch in range(NCH)]
        v_sb = [resid.tile([P, CHUNK], BF16, tag=f"v{ch}", name=f"v{ch}")
                for ch in range(NCH)]
        ot_sb = [[resid.tile([P, CHUNK], BF16, tag=f"ot{h}_{ch}", name=f"ot{h}_{ch}")
                  for ch in range(NCH)] for h in range(NHL)]

        def rope_hc(ps, raw_sw, out_tile, hc):
            """ps: (128, CHUNK//2) f32 PSUM [re; im]; raw_sw: bf16 SBUF with
            halves swapped [im; re] (produced by two ACT copies).
            out = ps*cosX + raw_sw*sinX with cosX = [cos; cos],
            sinX = [-sin; +sin]."""
            ch, half = hc // 2, hc % 2
            HC2 = CHUNK // 2
            cos = cs_sb[:, ch, 0, half * HC2:(half + 1) * HC2]
            sin = cs_sb[:, ch, 1, half * HC2:(half + 1) * HC2]
            t1 = tmppool.tile([P, HC2], F32, tag="t1", name="t1", bufs=3)
            t2 = tmppool.tile([P, HC2], F32, tag="t2", name="t2", bufs=3)
            nc.vector.tensor_mul(t1[:], ps[:], cos)
            nc.vector.tensor_mul(t2[:], raw_sw[:], sin)
            nc.vector.tensor_add(out_tile[:], t1[:], t2[:])

        HC = CHUNK // 2      # 256-wide half chunks: the qk PSUM footprint
        # is 3 banks (two heads packed per bank) + 1 shared V bank.
        vbank = [None]
        a_rest = {}          # hc -> deferred rope-rest generator (q1..q3)

        # global x sub-slab prefetcher: sub-slab i covers half-chunk i//4,
        # k-tiles (i%4)*8..(i%4)*8+7.  Consumption is strictly sequential
        # across A phases, so a single emitted-counter gives cross-phase
        # prefetch lead.
        NSUB = 2 * NCH * 4
        xstate = {"emitted": 0, "tiles": {}}

        def ensure_x(upto):
            while xstate["emitted"] < min(upto, NSUB):
                i = xstate["emitted"]
                t = xpool.tile([P, 8, HC], BF16, tag="xt", name="xt")
                nc.sync.dma_start(t[:], x5[i])
                xstate["tiles"][i] = t
                xstate["emitted"] = i + 1

        def gen_a(hc, defer=False):
            """qkv projection + RoPE for seq half-chunk hc (generator: one
            quantum per k-tile, then the rope epilogue).  With defer=True
            only the k and q0 rotations are emitted inline; q1..q3 are left
            in a_rest[hc] so a pre-started B head's counting-semaphore
            thresholds exclude them (its first scores need only k/q0)."""
            ch, half = hc // 2, hc % 2
            load_cs(ch)
            qk_bank = [psum.tile([P, CHUNK], F32, tag="ps", name="ps")
                       for _ in range((QKM + 1) // 2)]
            if half == 0:
                vbank[0] = psum.tile([P, CHUNK], F32, tag="ps", name="ps")
            ps_v = vbank[0]

            def qk_slice(m):
                return qk_bank[m // 2][:, (m % 2) * HC:(m % 2 + 1) * HC]

            for k in range(NKT):
                sub = hc * 4 + k // 8
                ensure_x(sub + 3)
                xt = xstate["tiles"][sub][:, k % 8]
                for m in range(QKM):
                    nc.tensor.matmul(
                        qk_slice(m), wqk_sb[k][:, m * P:(m + 1) * P], xt[:],
                        start=(k == 0 and m % 2 == 0),
                        stop=(k == NKT - 1 and (m % 2 == 1 or m == QKM - 1)),
                        skip_group_check=True)
                for t in range(2):
                    nc.tensor.matmul(
                        ps_v[:, (2 * half + t) * P:(2 * half + t + 1) * P],
                        xt[:, t * P:(t + 1) * P], wv_sb[k][:],
                        start=(half == 0 and k == 0 and t == 0),
                        stop=(half == 1 and k == NKT - 1 and t == 1),
                        skip_group_check=True)
                yield
            if half == 1:
                nc.scalar.copy(v_sb[ch][:], ps_v[:])
            rawsw = [tmppool.tile([P, HC], BF16, tag=f"qksw{m}", name=f"qksw{m}", bufs=2)
                     for m in range(QKM)]
            order = [NHL] + list(range(NHL))     # k tile first
            for m in order:
                nc.scalar.copy(rawsw[m][0:64, :], qk_slice(m)[64:128, :])
                nc.scalar.copy(rawsw[m][64:128, :], qk_slice(m)[0:64, :])
            yield

            def rope_m(m):
                out_tile = k_sb[ch] if m == NHL else q_sb[m][ch]
                rope_hc(qk_slice(m), rawsw[m],
                        out_tile[:, half * HC:(half + 1) * HC], hc)

            for m in (NHL, 0):
                rope_m(m)
                yield
            if defer:
                def rest():
                    for m in (1, 2, 3):
                        rope_m(m)
                        yield
                a_rest[hc] = rest()
                return
            for m in (1, 2, 3):
                rope_m(m)
                yield

        def gen_a_full(ch):
            """full-chunk (512-wide) A phase -- only legal while no B/C
            phase is interleaved (needs 6 PSUM banks).  Used for chunk 0:
            fewer, larger matmuls give the startup weight stream 2x more
            JIT slack and cut instruction overhead."""
            load_cs(ch)
            qk_bank = [psum.tile([P, CHUNK], F32, tag="ps", name="ps")
                       for _ in range(QKM)]
            ps_v = psum.tile([P, CHUNK], F32, tag="ps", name="ps")
            for k in range(NKT):
                # chunk 0's sub-slabs are host-ordered interleaved:
                # [h0s0, h1s0, h0s1, h1s1, ...] to match this loop.
                pair = ch * 8 + 2 * (k // 8)
                ensure_x(pair + 4)
                for half in (0, 1):
                    xt = xstate["tiles"][pair + half][:, k % 8]
                    for m in range(QKM):
                        nc.tensor.matmul(
                            qk_bank[m][:, half * HC:(half + 1) * HC],
                            wqk_sb[k][:, m * P:(m + 1) * P], xt[:],
                            start=(k == 0 and half == 0),
                            stop=(k == NKT - 1 and half == 1),
                            skip_group_check=True)
                    for t in range(2):
                        nc.tensor.matmul(
                            ps_v[:, (2 * half + t) * P:(2 * half + t + 1) * P],
                            xt[:, t * P:(t + 1) * P], wv_sb[k][:],
                            start=(k == 0 and t == 0 and half == 0),
                            stop=(k == NKT - 1 and t == 1 and half == 1),
                            skip_group_check=True)
                if ch == 0 and k == 7:
                    emit_wdma(4, len(WGROUPS))
                yield
            nc.scalar.copy(v_sb[ch][:], ps_v[:])
            rawsw = [tmppool.tile([P, CHUNK], BF16, tag=f"qksf{m}",
                                  name=f"qksf{m}", bufs=1)
                     for m in range(QKM)]
            for m in [NHL] + list(range(NHL)):
                nc.scalar.copy(rawsw[m][0:64, :], qk_bank[m][64:128, :])
                nc.scalar.copy(rawsw[m][64:128, :], qk_bank[m][0:64, :])
            yield
            for m in [NHL] + list(range(NHL)):
                out_tile = k_sb[ch] if m == NHL else q_sb[m][ch]
                cos = cs_sb[:, ch, 0, :]
                sin = cs_sb[:, ch, 1, :]
                t1 = tmppool.tile([P, CHUNK], F32, tag="tf1", name="tf1", bufs=2)
                t2 = tmppool.tile([P, CHUNK], F32, tag="tf2", name="tf2", bufs=2)
                nc.vector.tensor_mul(t1[:], qk_bank[m][:], cos)
                nc.vector.tensor_mul(t2[:], rawsw[m][:], sin)
                nc.vector.tensor_add(out_tile[:], t1[:], t2[:])
                yield

        def gen_b(ch, lookahead=False):
            """attention for all local heads, q chunk ch (causal).  One
            quantum per (head, k-tile) step; the driver inserts an
            independent PE quantum in each gap so exp never stalls the PE.
            With lookahead=True the j+1 score matmul is also emitted before
            the exp-dependent sums/PV of j (used when little filler is
            available)."""
            njt = 4 * ch + 4

            def score(h, j):
                o = j - 4 * ch          # >=0: diagonal region, trim N
                lo = max(o, 0) * P      # first valid q column
                ps_st = psum.tile([P, CHUNK], F32, tag="ps", name="ps")
                nc.tensor.matmul(
                    ps_st[:, lo:], k_sb[j // 4][:, (j % 4) * P:(j % 4 + 1) * P],
                    q_sb[h][ch][:, lo:], start=True, stop=True,
                    skip_group_check=True)
                pt = ptpool.tile([P, CHUNK], BF16, tag="pt", name="pt")
                if o >= 0:  # mask the diagonal 128x128 block
                    nc.vector.tensor_add(
                        ps_st[:, o * P:(o + 1) * P],
                        ps_st[:, o * P:(o + 1) * P], mask_sb[:])
                nc.scalar.activation(
                    pt[:, lo:], ps_st[:, lo:],
                    mybir.ActivationFunctionType.Exp, bias=negC[:])
                return pt, lo

            for h in range(NHL):
                ps_sum = psum.tile([P, CHUNK], F32, tag="ps", name="ps")
                ps_ot = psum.tile([P, CHUNK], F32, tag="ps", name="ps")
                nxt = score(h, 0)
                for j in range(njt):
                    pt, lo = nxt
                    if lookahead and j + 1 < njt:
                        nxt = score(h, j + 1)
                    yield               # filler slot: exp(pt_j) runs here
                    if not lookahead and j + 1 < njt:
                        nxt = score(h, j + 1)
                    nc.tensor.matmul(ps_sum[:, lo:], ones_sb[:], pt[:, lo:],
                                     start=(j == 0), stop=(j == njt - 1),
                                     skip_group_check=True)
                    nc.tensor.matmul(
                        ps_ot[:, lo:], v_sb[j // 4][:, (j % 4) * P:(j % 4 + 1) * P],
                        pt[:, lo:], start=(j == 0), stop=(j == njt - 1),
                        skip_group_check=True)
                recip = tmppool.tile([P, CHUNK], F32, tag="recip", name="recip", bufs=2)
                nc.vector.reciprocal_approx_fast(out=recip[:], in_=ps_sum[:])
                nc.vector.tensor_mul(ot_sb[h][ch][:], ps_ot[:], recip[:])
                yield

        def gen_c(ch, dlo=0, dhi=None, tail=False, sync_ok=False,
                  evict_act=False):
            """output projection for the 4 seq tiles of chunk ch, output dim
            chunks dlo..dhi (generator: one quantum per (t, d) tile).
            While any B phase is still running (non-tail), PSUM evictions go
            DVE-only -- an ACT eviction queued between exps would stall the
            exp-dependent matmuls -- and out-DMAs ride gpsimd (+sync once no
            x fetches follow).  At the true tail ACT is free: alternate
            ACT/DVE and rotate three DMA queues."""
            if dhi is None:
                dhi = NDC
            if tail:
                qs = [nc.sync, nc.gpsimd, nc.scalar]
            elif sync_ok:        # no x fetches follow: sync is HOL-safe
                qs = [nc.sync, nc.gpsimd]
            else:
                qs = [nc.gpsimd]
            qi = 0
            for tq in range(4):
                t = 4 * ch + tq
                for d in range(dlo, dhi):
                    ps_o = psum.tile([P, CHUNK], F32, tag="ps", name="ps")
                    for h in range(NHL):
                        nc.tensor.matmul(
                            ps_o[:], ot_sb[h][ch][:, tq * P:(tq + 1) * P],
                            wo_sb[:, h, d * CHUNK:(d + 1) * CHUNK],
                            start=(h == 0), stop=(h == NHL - 1),
                            skip_group_check=True)
                    ob = obpool.tile([P, CHUNK], BF16, tag="ob", name="ob")
                    if (tail and d % 2 == 1) or evict_act:
                        nc.scalar.copy(ob[:], ps_o[:])
                    else:
                        nc.vector.tensor_scalar_mul(ob[:], ps_o[:], 1.0)
                    if tail and tq == 3:
                        # final seq tile: halve each out-DMA across two
                        # queues so the post-compute drain chain (eviction
                        # -> DMA wire -> completion) parallelizes.
                        for piece in range(2):
                            sl = slice((d * 2 + piece) * (CHUNK // 2),
                                       (d * 2 + piece + 1) * (CHUNK // 2))
                            qs[qi % len(qs)].dma_start(
                                out[t * P:(t + 1) * P, sl],
                                ob[:, piece * (CHUNK // 2):
                                   (piece + 1) * (CHUNK // 2)])
                            qi += 1
                    else:
                        qs[qi % len(qs)].dma_start(
                            out[t * P:(t + 1) * P,
                                d * CHUNK:(d + 1) * CHUNK], ob[:])
                        qi += 1
                    yield

        def run(gen):
            for _ in gen:
                pass

        def take(gen, n):
            """yield up to n quanta from a shared generator."""
            for _ in range(n):
                if next(gen, "__done__") == "__done__":
                    return
                yield

        def chain(*gens):
            for g in gens:
                yield from g

        def rest(hc):
            """deferred rope quanta of gen_a(hc, defer=True), resolved
            lazily (the generator exists only once gen_a(hc) finished)."""
            yield from a_rest.pop(hc)

        def mix(*gens):
            """round-robin one quantum at a time until all are exhausted."""
            live = list(gens)
            while live:
                for g in list(live):
                    if next(g, "__done__") == "__done__":
                        live.remove(g)
                    else:
                        yield

        def interleave(*pairs):
            """pairs: (generator, weight).  Weighted fair queueing at
            single-quantum granularity: each step emits one quantum from the
            generator with the highest accumulated credit, so any ratio
            interleaves smoothly instead of in bursts."""
            state = [[g, float(w), 0.0] for g, w in pairs]
            while state:
                tot = sum(st[1] for st in state)
                for st in state:
                    st[2] += st[1] / tot
                st = max(state, key=lambda s: s[2])
                st[2] -= 1.0
                if next(st[0], "__done__") == "__done__":
                    state.remove(st)

        # ---- schedule ----
        # A(hc) covers chunk hc//2; B(b) needs chunks <= b roped and v'd;
        # C(b) needs all of B(b).  Fillers keep exp off the PE critical
        # path.  Each B's head is pre-started inside the previous C
        # interleave (after the A epilogue that ropes its chunk) so its
        # counting-semaphore thresholds exclude that C's final evictions.
        b1 = gen_b(1)
        b2 = gen_b(2)
        b3 = gen_b(3, lookahead=True)
        run(gen_a_full(0))
        a2 = gen_a(2)
        run(take(a2, 6))     # cover chunk 0's rope latency (DVE) with A2
                             # matmuls before B0's rope-dependent scores join
        interleave((gen_b(0), 1), (a2, 2))
        # wo streams here (11us of wire), after the startup x/weight crunch
        # but a full interleave ahead of its first phase-C consumer.
        load_wo(0)
        load_wo(1)
        interleave((gen_c(0), 2), (chain(gen_a(3), take(b1, 9)), 3))
        interleave((b1, 2), (gen_a(4), 3))
        interleave((gen_c(1), 4), (chain(gen_a(5), take(b2, 13)), 7))
        interleave((b2, 1), (gen_a(6), 1))
        interleave((gen_c(2, 0, 6, evict_act=True), 3),
                   (chain(gen_a(7), take(b3, 20)), 8))
        interleave((b3, 6), (gen_c(2, 6, 8, sync_ok=True), 1))
        run(gen_c(3, tail=True))

    return nc


def _make_in_maps(x, freqs_cis, wqkv, wo):
    scale = np.float32(1.0 / np.sqrt(HD))
    xT = np.asarray(x)[0].T.astype(NPBF16)               # (DIM, S)
    # x sub-slabs (32, 128, 8, 256): slab hc*4+sub holds k-tiles sub*8..+7
    # (partition-major) for half-chunk hc -- each is one contiguous DMA.
    xkt = xT.reshape(32, 128, S)
    # chunk 0 (half-chunks 0,1) interleaved [h0s0, h1s0, h0s1, ...] for the
    # full-chunk A phase; chunks 1-3 sequential per half-chunk.
    order = [(hc, sub) for sub in range(4) for hc in (0, 1)] + \
            [(hc, sub) for hc in range(2, 8) for sub in range(4)]
    x5 = np.ascontiguousarray(np.stack(
        [xkt[sub * 8:(sub + 1) * 8, :, hc * 256:(hc + 1) * 256].transpose(1, 0, 2)
         for hc, sub in order]))
    NCH, CHUNK = 4, 512
    cos = freqs_cis[:, :, 0].T.astype(np.float32)        # (64, S)
    sin = freqs_cis[:, :, 1].T.astype(np.float32)
    cosT = np.concatenate([cos, cos], 0)                 # (128, S)
    sinT = np.concatenate([-sin, sin], 0)
    # (128, NCH, 2, CHUNK): per chunk one contiguous cos||sin slab
    csT = np.ascontiguousarray(
        np.stack([cosT.reshape(128, NCH, CHUNK),
                  sinT.reshape(128, NCH, CHUNK)], axis=2))
    ones = np.ones((128, 128), NPBF16)
    kp = np.arange(128)[:, None]
    qp = np.arange(128)[None, :]
    maskT = np.where(kp <= qp, 0.0, -1e30).astype(np.float32)

    in_maps = []
    for c in range(N_CORES):
        rows = [wqkv[128 * (NHL * c + h) + PERM] * scale for h in range(NHL)]
        rows.append(wqkv[NH * HD + 128 * c + PERM])
        wqkT = np.concatenate(rows, 0).T                  # (DIM, 640)
        wvT = wqkv[(NH + NKV) * HD + 128 * c:(NH + NKV) * HD + 128 * (c + 1)].T
        wqkvT = np.concatenate([wqkT, wvT], 1)            # (DIM, 768)
        # pre-swizzle (kt*128+p, m) -> (p, kt, m): device group loads become
        # contiguous slab reads instead of strided gathers
        wqkvT = np.ascontiguousarray(
            wqkvT.reshape(32, 128, -1).transpose(1, 0, 2)).astype(NPBF16)
        woT = wo[:, 128 * NHL * c:128 * NHL * (c + 1)].T  # (512, DIM)
        woT = np.ascontiguousarray(
            woT.reshape(NHL, 128, DIM).transpose(1, 0, 2)).astype(NPBF16)
        in_maps.append({
            "x5": x5, "wqkvT": wqkvT, "woT": woT,
            "csT": csT, "onesW": ones, "maskT": maskT,
        })
    return in_maps


def kernel(x, freqs_cis, wqkv, wo):
    x = np.asarray(x, dtype=np.float32)
    freqs_cis = np.asarray(freqs_cis, dtype=np.float32)
    wqkv = np.asarray(wqkv, dtype=np.float32)
    wo = np.asarray(wo, dtype=np.float32)

    in_maps = _make_in_maps(x, freqs_cis, wqkv, wo)
    nc = bacc.Bacc("TRN2", target_bir_lowering=False, debug=False,
                   num_devices=N_CORES)
    build_attention_kernel(nc, S=S, DIM=DIM)
    nc.compile()
    res = run_bass_kernel_spmd(nc, in_maps, core_ids=list(range(N_CORES)))

    acc = np.zeros((S, DIM), np.float32)
    for r in res.results:
        acc += np.asarray(r["out"]).astype(np.float32)
    return acc[None]


# revision 52
# speedup vs baseline: 1.1977x; 1.1977x over previous
"""Tensor-parallel GQA attention block on 8 TRN2 NeuronCores (Bass/Tile).

Problem: B=1, S=2048, DIM=4096, 32 q heads / 8 kv heads (GQA), head_dim=128,
RoPE, causal softmax, output projection.

Sharding (tensor parallel by head, per the hint): core c of 8 owns q heads
4c..4c+3 and kv head c (GQA groups stay with their q heads). wqkv rows and wo
columns are sharded by head; attention is fully local per core; each core
emits a partial (S, DIM) output (its heads through its wo column slice) and
the partials are summed on the host at unshard time (the "all-reduce after
wo" of the hint, done off-device since full I/O passes through the host
anyway).

Per-core device kernel -- all operands host-pre-transposed so every matmul has
its contraction dim on SBUF partitions; zero on-device transposes:
  qkT = wqkT.T @ xT              (head dims on partitions, seq free)
  v   = xT.T @ wvT               (seq on partitions, head dim free)
  RoPE on qT/kT in transposed layout: host permutes rows into re(0..63)/
    im(64..127); cos/sin arrive as stacked (128, S) tables [cos;cos] and
    [-sin;sin]; 1/sqrt(HD) is folded into wq on the host.
  per head, per 512-wide q chunk (causal: only k tiles <= chunk end):
    S.T[j] = kT_j.T @ qT_chunk   (k positions on partitions => softmax
                                  denominators via a ones-matmul; no P
                                  transpose anywhere)
    P.T[j] = exp(S.T[j] - 12)    (triangular mask added on diagonal tiles;
                                  N trimmed to the causal columns)
    sums  += ones128.T @ P.T[j]  (PSUM-accumulated, rows replicated)
    O.T   += matmul(lhsT=V_j, rhs=P.T[j])
    O.T_norm = O.T * reciprocal_approx(sums)  -> bf16
  out[t, d] = sum_h O.T_h[:, t].T @ woT_h[:, d]

Schedule: quantum-interleaved emission.  Each phase is a generator of PE
"quanta" (~0.6-0.9us of matmuls); a weighted-fair driver mixes them so that
during attention the exp-dependent ones/PV matmuls always have independent
projection/output quanta between them and their score matmul -- the ACT
engine's exp latency hides under PE work instead of stalling it.  Each B
phase's first j-steps are pre-started inside the previous C interleave
(shared generator) so their counting-semaphore thresholds exclude C's
final PSUM evictions.  Weights stream as 9 staggered group DMAs of a
combined wqkv tensor (SWDGE descriptor generation is ~0.7us per DMA, so
descriptor count, not just bytes, gates startup); wo and per-chunk cos/sin
slices load outside the startup window.  PSUM->SBUF evictions alternate
ACT/DVE and out-DMAs rotate across engine queues (4-way at the tail).

Compute in bf16 with f32 PSUM accumulation; rel l2 error vs the f32
reference is ~8e-3.
"""
import sys

sys.path.insert(0, "/opt/trn_rl_repo")

from contextlib import ExitStack

import numpy as np
import ml_dtypes

import concourse.bass as bass
import concourse.tile as tile
import concourse.mybir as mybir
from concourse import bacc
from concourse.bass_utils import run_bass_kernel_spmd

F32 = mybir.dt.float32
BF16 = mybir.dt.bfloat16
NPBF16 = ml_dtypes.bfloat16

NH, NKV, HD = 32, 8, 128
S, DIM = 2048, 4096
N_CORES = 8
NHL = NH // N_CORES          # q heads per core
PERM = np.concatenate([np.arange(0, 128, 2), np.arange(1, 128, 2)])
WGROUPS = [2, 2, 4, 4, 4, 4, 4, 4, 4]   # k-tiles per weight-group DMA


def build_attention_kernel(nc, S=2048, DIM=4096, C=12.0):
    NHL = 4          # local q heads
    HD = 128
    CHUNK = 512
    P = 128
    NKT = DIM // P         # k tiles over model dim
    NCH = S // CHUNK       # seq chunks
    QKM = NHL + 1          # m-tiles in qk GEMM (4 q heads + 1 k head)
    NDC = DIM // CHUNK     # output dim chunks
    WM = QKM * P + HD      # combined wqkv row width (640 qk + 128 v)

    # ---- DRAM I/O ----
    # x, host pre-swizzled into 32 contiguous 512KB sub-slabs (8 k-tiles x
    # one 256-col half-chunk each, partition-major): one DMA instruction per
    # sub-slab instead of one per (k, hc) tile -- the HWDGE queue's ~0.66us
    # per-instruction cost made per-tile x fetches the feeding bottleneck.
    x5 = nc.dram_tensor("x5", (2 * (S // 512) * 4, P, 8, 256), BF16,
                        kind="ExternalInput").ap()
    # combined qkv weights, host pre-swizzled to (p, kt, m): columns 0..639 =
    # wqkT (4 q heads + 1 k head), 640..767 = wvT -- each k-tile-group DMA is
    # a contiguous slab read (a strided (kt p)->p kt gather costs far more
    # SWDGE descriptor time and delays the first matmul).
    wqkvT = nc.dram_tensor("wqkvT", (P, NKT, WM), BF16,
                           kind="ExternalInput").ap()
    woT = nc.dram_tensor("woT", (P, NHL, DIM), BF16, kind="ExternalInput").ap()
    # csT[:, ch, 0, :] = cos columns of chunk ch ([cos;cos] stacked rows),
    # csT[:, ch, 1, :] = sin columns ([-sin;+sin]) -- one DMA per chunk.
    csT = nc.dram_tensor("csT", (128, NCH, 2, CHUNK), F32,
                         kind="ExternalInput").ap()
    onesW = nc.dram_tensor("onesW", (P, P), BF16, kind="ExternalInput").ap()
    maskT = nc.dram_tensor("maskT", (P, P), F32, kind="ExternalInput").ap()
    out = nc.dram_tensor("out", (S, DIM), BF16, kind="ExternalOutput").ap()

    with tile.TileContext(nc) as tc, ExitStack() as ctx:
        const = ctx.enter_context(tc.tile_pool(name="const", bufs=1))
        resid = ctx.enter_context(tc.tile_pool(name="resid", bufs=1))
        xpool = ctx.enter_context(tc.tile_pool(name="xp", bufs=5))
        ptpool = ctx.enter_context(tc.tile_pool(name="ptp", bufs=6))
        tmppool = ctx.enter_context(tc.tile_pool(name="tmp", bufs=4))
        obpool = ctx.enter_context(tc.tile_pool(name="obp", bufs=8))
        psum = ctx.enter_context(tc.tile_pool(name="psum", bufs=8, space="PSUM"))

        # ---- weights: 9 staggered group DMAs (small groups first so the
        # first matmul starts in ~2us, larger ones amortize the per-DMA
        # SWDGE descriptor cost while the wire streams). ----
        w_g = []
        k0 = 0
        for gi, gn in enumerate(WGROUPS):
            g = const.tile([P, gn, WM], BF16, tag=f"wg{gi}", name=f"wg{gi}")
            w_g.append((g, k0, gn, gi))
            k0 += gn

        def emit_wdma(lo, hi):
            for g, k0, gn, gi in w_g[lo:hi]:
                # alternate queues: the ~4.5us first-DMA latency and the
                # wire time are paid in parallel across two otherwise-idle
                # queues.
                eng = nc.scalar if gi % 2 == 0 else nc.gpsimd
                eng.dma_start(g[:], wqkvT[:, k0:k0 + gn, :])

        # only the first 4 groups (k-tiles 0..11) stream up front -- the
        # rest would steal wire bandwidth from the first x slabs; they are
        # emitted from inside gen_a_full once the pipeline is rolling.
        emit_wdma(0, 4)
        ktile = []
        for g, k0, gn, gi in w_g:
            for i in range(gn):
                ktile.append(g[:, i])
        wqk_sb = [ktile[k][:, 0:QKM * P] for k in range(NKT)]
        wv_sb = [ktile[k][:, QKM * P:WM] for k in range(NKT)]
        ones_sb = const.tile([P, P], BF16, tag="ones", name="ones")
        nc.gpsimd.dma_start(ones_sb[:], onesW[:])
        mask_sb = const.tile([P, P], F32, tag="mask", name="mask")
        nc.gpsimd.dma_start(mask_sb[:], maskT[:])
        negC = const.tile([P, 1], F32, tag="negC", name="negC")
        nc.any.memset(negC[:], -C)
        cs_sb = const.tile([P, NCH, 2, CHUNK], F32, tag="cs", name="cs")
        wo_sb = const.tile([P, NHL, DIM], BF16, tag="wo", name="wo")

        cs_loaded = [False] * NCH

        def load_cs(ch):
            """cos/sin columns for chunk ch -- emitted at the start of the
            chunk's first A half so the slice has a full phase to arrive."""
            if cs_loaded[ch]:
                return
            cs_loaded[ch] = True
            nc.gpsimd.dma_start(cs_sb[:, ch], csT[:, ch])

        load_cs(0)
        load_cs(1)

        def load_wo(half):
            sl = slice(half * (DIM // 2), (half + 1) * (DIM // 2))
            nc.gpsimd.dma_start(wo_sb[:, :, sl], woT[:, :, sl])

        # resident activations (per chunk tiles for fine-grained deps)
        q_sb = [[resid.tile([P, CHUNK], BF16, tag=f"q{h}_{ch}", name=f"q{h}_{ch}")
                 for ch in range(NCH)] for h in range(NHL)]
        k_sb = [resid.tile([P, CHUNK], BF16, tag=f"k{ch}", name=f"k{ch}")
                for ch in range(NCH)]
        v_sb = [resid.tile([P, CHUNK], BF16, tag=f"v{ch}", name=f"v{ch}")
                for ch in range(NCH)]
        ot_sb = [[resid.tile([P, CHUNK], BF16, tag=f"ot{h}_{ch}", name=f"ot{h}_{ch}")
                  for ch in range(NCH)] for h in range(NHL)]

        def rope_hc(ps, raw_sw, out_tile, hc):
            """ps: (128, CHUNK//2) f32 PSUM [re; im]; raw_sw: bf16 SBUF with
            halves swapped [im; re] (produced by two ACT copies).
            out = ps*cosX + raw_sw*sinX with cosX = [cos; cos],
            sinX = [-sin; +sin]."""
            ch, half = hc // 2, hc % 2
            HC2 = CHUNK // 2
            cos = cs_sb[:, ch, 0, half * HC2:(half + 1) * HC2]
            sin = cs_sb[:, ch, 1, half * HC2:(half + 1) * HC2]
            t1 = tmppool.tile([P, HC2], F32, tag="t1", name="t1", bufs=3)
            t2 = tmppool.tile([P, HC2], F32, tag="t2", name="t2", bufs=3)
            nc.vector.tensor_mul(t1[:], ps[:], cos)
            nc.vector.tensor_mul(t2[:], raw_sw[:], sin)
            nc.vector.tensor_add(out_tile[:], t1[:], t2[:])

        HC = CHUNK // 2      # 256-wide half chunks: the qk PSUM footprint
        # is 3 banks (two heads packed per bank) + 1 shared V bank.
        vbank = [None]
        a_rest = {}          # hc -> deferred rope-rest generator (q1..q3)

        # global x sub-slab prefetcher: sub-slab i covers half-chunk i//4,
        # k-tiles (i%4)*8..(i%4)*8+7.  Consumption is strictly sequential
        # across A phases, so a single emitted-counter gives cross-phase
        # prefetch lead.
        NSUB = 2 * NCH * 4
        xstate = {"emitted": 0, "tiles": {}}

        def ensure_x(upto):
            while xstate["emitted"] < min(upto, NSUB):
                i = xstate["emitted"]
                t = xpool.tile([P, 8, HC], BF16, tag="xt", name="xt")
                nc.sync.dma_start(t[:], x5[i])
                xstate["tiles"][i] = t
                xstate["emitted"] = i + 1

        def gen_a(hc, defer=False):
            """qkv projection + RoPE for seq half-chunk hc (generator: one
            quantum per k-tile, then the rope epilogue).  With defer=True
            only the k and q0 rotations are emitted inline; q1..q3 are left
            in a_rest[hc] so a pre-started B head's counting-semaphore
            thresholds exclude them (its first scores need only k/q0)."""
            ch, half = hc // 2, hc % 2
            load_cs(ch)
            qk_bank = [psum.tile([P, CHUNK], F32, tag="ps", name="ps")
                       for _ in range((QKM + 1) // 2)]
            if half == 0:
                vbank[0] = psum.tile([P, CHUNK], F32, tag="ps", name="ps")
            ps_v = vbank[0]

            def qk_slice(m):
                return qk_bank[m // 2][:, (m % 2) * HC:(m % 2 + 1) * HC]

            for k in range(NKT):
                sub = hc * 4 + k // 8
                ensure_x(sub + 3)
                xt = xstate["tiles"][sub][:, k % 8]
                for m in range(QKM):
                    nc.tensor.matmul(
                        qk_slice(m), wqk_sb[k][:, m * P:(m + 1) * P], xt[:],
                        start=(k == 0 and m % 2 == 0),
                        stop=(k == NKT - 1 and (m % 2 == 1 or m == QKM - 1)),
                        skip_group_check=True)
                for t in range(2):
                    nc.tensor.matmul(
                        ps_v[:, (2 * half + t) * P:(2 * half + t + 1) * P],
                        xt[:, t * P:(t + 1) * P], wv_sb[k][:],
                        start=(half == 0 and k == 0 and t == 0),
                        stop=(half == 1 and k == NKT - 1 and t == 1),
                        skip_group_check=True)
                yield
            if half == 1:
                nc.scalar.copy(v_sb[ch][:], ps_v[:])
            rawsw = [tmppool.tile([P, HC], BF16, tag=f"qksw{m}", name=f"qksw{m}", bufs=2)
                     for m in range(QKM)]
            order = [NHL] + list(range(NHL))     # k tile first
            for m in order:
                nc.scalar.copy(rawsw[m][0:64, :], qk_slice(m)[64:128, :])
                nc.scalar.copy(rawsw[m][64:128, :], qk_slice(m)[0:64, :])
            yield

            def rope_m(m):
                out_tile = k_sb[ch] if m == NHL else q_sb[m][ch]
                rope_hc(qk_slice(m), rawsw[m],
                        out_tile[:, half * HC:(half + 1) * HC], hc)

            for m in (NHL, 0):
                rope_m(m)
                yield
            if defer:
                def rest():
                    for m in (1, 2, 3):
                        rope_m(m)
                        yield
                a_rest[hc] = rest()
                return
            for m in (1, 2, 3):
                rope_m(m)
                yield

        def gen_a_full(ch):
            """full-chunk (512-wide) A phase -- only legal while no B/C
            phase is interleaved (needs 6 PSUM banks).  Used for chunk 0:
            fewer, larger matmuls give the startup weight stream 2x more
            JIT slack and cut instruction overhead."""
            load_cs(ch)
            qk_bank = [psum.tile([P, CHUNK], F32, tag="ps", name="ps")
                       for _ in range(QKM)]
            ps_v = psum.tile([P, CHUNK], F32, tag="ps", name="ps")
            for k in range(NKT):
                # chunk 0's sub-slabs are host-ordered interleaved:
                # [h0s0, h1s0, h0s1, h1s1, ...] to match this loop.
                pair = ch * 8 + 2 * (k // 8)
                ensure_x(pair + 4)
                for half in (0, 1):
                    xt = xstate["tiles"][pair + half][:, k % 8]
                    for m in range(QKM):
                        nc.tensor.matmul(
                            qk_bank[m][:, half * HC:(half + 1) * HC],
                            wqk_sb[k][:, m * P:(m + 1) * P], xt[:],
                            start=(k == 0 and half == 0),
                            stop=(k == NKT - 1 and half == 1),
                            skip_group_check=True)
                    for t in range(2):
                        nc.tensor.matmul(
                            ps_v[:, (2 * half + t) * P:(2 * half + t + 1) * P],
                            xt[:, t * P:(t + 1) * P], wv_sb[k][:],
                            start=(k == 0 and t == 0 and half == 0),
                            stop=(k == NKT - 1 and t == 1 and half == 1),
                            skip_group_check=True)
                if ch == 0 and k == 7:
                    emit_wdma(4, len(WGROUPS))
                yield
            nc.scalar.copy(v_sb[ch][:], ps_v[:])
            rawsw = [tmppool.tile([P, CHUNK], BF16, tag=f"qksf{m}",
                                  name=f"qksf{m}", bufs=1)
                     for m in range(QKM)]
            for m in [NHL] + list(range(NHL)):
                nc.scalar.copy(rawsw[m][0:64, :], qk_bank[m][64:128, :])
                nc.scalar.copy(rawsw[m][64:128, :], qk_bank[m][0:64, :])
            yield
            for m in [NHL] + list(range(NHL)):
                out_tile = k_sb[ch] if m == NHL else q_sb[m][ch]
                cos = cs_sb[:, ch, 0, :]
                sin = cs_sb[:, ch, 1, :]
                t1 = tmppool.tile([P, CHUNK], F32, tag="tf1", name="tf1", bufs=2)
                t2 = tmppool.tile([P, CHUNK], F32, tag="tf2", name="tf2", bufs=2)
                nc.vector.tensor_mul(t1[:], qk_bank[m][:], cos)
                nc.vector.tensor_mul(t2[:], rawsw[m][:], sin)
                nc.vector.tensor_add(out_tile[:], t1[:], t2[:])
                yield

        def gen_b(ch, lookahead=False):
            """attention for all local heads, q chunk ch (causal).  One
            quantum per (head, k-tile) step; the driver inserts an
            independent PE quantum in each gap so exp never stalls the PE.
            With lookahead=True the j+1 score matmul is also emitted before
            the exp-dependent sums/PV of j (used when little filler is
            available)."""
            njt = 4 * ch + 4

            def score(h, j):
                o = j - 4 * ch          # >=0: diagonal region, trim N
                lo = max(o, 0) * P      # first valid q column
                ps_st = psum.tile([P, CHUNK], F32, tag="ps", name="ps")
                nc.tensor.matmul(
                    ps_st[:, lo:], k_sb[j // 4][:, (j % 4) * P:(j % 4 + 1) * P],
                    q_sb[h][ch][:, lo:], start=True, stop=True,
                    skip_group_check=True)
                pt = ptpool.tile([P, CHUNK], BF16, tag="pt", name="pt")
                if o >= 0:  # mask the diagonal 128x128 block
                    nc.vector.tensor_add(
                        ps_st[:, o * P:(o + 1) * P],
                        ps_st[:, o * P:(o + 1) * P], mask_sb[:])
                nc.scalar.activation(
                    pt[:, lo:], ps_st[:, lo:],
                    mybir.ActivationFunctionType.Exp, bias=negC[:])
                return pt, lo

            for h in range(NHL):
                ps_sum = psum.tile([P, CHUNK], F32, tag="ps", name="ps")
                ps_ot = psum.tile([P, CHUNK], F32, tag="ps", name="ps")
                nxt = score(h, 0)
                for j in range(njt):
                    pt, lo = nxt
                    if lookahead and j + 1 < njt:
                        nxt = score(h, j + 1)
                    yield               # filler slot: exp(pt_j) runs here
                    if not lookahead and j + 1 < njt:
                        nxt = score(h, j + 1)
                    nc.tensor.matmul(ps_sum[:, lo:], ones_sb[:], pt[:, lo:],
                                     start=(j == 0), stop=(j == njt - 1),
                                     skip_group_check=True)
                    nc.tensor.matmul(
                        ps_ot[:, lo:], v_sb[j // 4][:, (j % 4) * P:(j % 4 + 1) * P],
                        pt[:, lo:], start=(j == 0), stop=(j == njt - 1),
                        skip_group_check=True)
                recip = tmppool.tile([P, CHUNK], F32, tag="recip", name="recip", bufs=2)
                nc.vector.reciprocal_approx_fast(out=recip[:], in_=ps_sum[:])
                nc.vector.tensor_mul(ot_sb[h][ch][:], ps_ot[:], recip[:])
                yield

        def gen_c(ch, dlo=0, dhi=None, tail=False, sync_ok=False,
                  evict_act=False):
            """output projection for the 4 seq tiles of chunk ch, output dim
            chunks dlo..dhi (generator: one quantum per (t, d) tile).
            While any B phase is still running (non-tail), PSUM evictions go
            DVE-only -- an ACT eviction queued between exps would stall the
            exp-dependent matmuls -- and out-DMAs ride gpsimd (+sync once no
            x fetches follow).  At the true tail ACT is free: alternate
            ACT/DVE and rotate three DMA queues."""
            if dhi is None:
                dhi = NDC
            if tail:
                qs = [nc.sync, nc.gpsimd, nc.scalar]
            elif sync_ok:        # no x fetches follow: sync is HOL-safe
                qs = [nc.sync, nc.gpsimd]
            else:
                qs = [nc.gpsimd]
            qi = 0
            for tq in range(4):
                t = 4 * ch + tq
                for d in range(dlo, dhi):
                    ps_o = psum.tile([P, CHUNK], F32, tag="ps", name="ps")
                    for h in range(NHL):
                        nc.tensor.matmul(
                            ps_o[:], ot_sb[h][ch][:, tq * P:(tq + 1) * P],
                            wo_sb[:, h, d * CHUNK:(d + 1) * CHUNK],
                            start=(h == 0), stop=(h == NHL - 1),
                            skip_group_check=True)
                    ob = obpool.tile([P, CHUNK], BF16, tag="ob", name="ob")
                    if (tail and d % 2 == 1) or evict_act:
                        nc.scalar.copy(ob[:], ps_o[:])
                    else:
                        nc.vector.tensor_scalar_mul(ob[:], ps_o[:], 1.0)
                    if tail and tq == 3:
                        # final seq tile: halve each out-DMA across two
                        # queues so the post-compute drain chain (eviction
                        # -> DMA wire -> completion) parallelizes.
                        for piece in range(2):
                            sl = slice((d * 2 + piece) * (CHUNK // 2),
                                       (d * 2 + piece + 1) * (CHUNK // 2))
                            qs[qi % len(qs)].dma_start(
                                out[t * P:(t + 1) * P, sl],
                                ob[:, piece * (CHUNK // 2):
                                   (piece + 1) * (CHUNK // 2)])
                            qi += 1
                    else:
                        qs[qi % len(qs)].dma_start(
                            out[t * P:(t + 1) * P,
                                d * CHUNK:(d + 1) * CHUNK], ob[:])
                        qi += 1
                    yield

        def run(gen):
            for _ in gen:
                pass

        def take(gen, n):
            """yield up to n quanta from a shared generator."""
            for _ in range(n):
                if next(gen, "__done__") == "__done__":
                    return
                yield

        def chain(*gens):
            for g in gens:
                yield from g

        def rest(hc):
            """deferred rope quanta of gen_a(hc, defer=True), resolved
            lazily (the generator exists only once gen_a(hc) finished)."""
            yield from a_rest.pop(hc)

        def mix(*gens):
            """round-robin one quantum at a time until all are exhausted."""
            live = list(gens)
            while live:
                for g in list(live):
                    if next(g, "__done__") == "__done__":
                        live.remove(g)
                    else:
                        yield

        def interleave(*pairs):
            """pairs: (generator, weight).  Weighted fair queueing at
            single-quantum granularity: each step emits one quantum from the
            generator with the highest accumulated credit, so any ratio
            interleaves smoothly instead of in bursts."""
            state = [[g, float(w), 0.0] for g, w in pairs]
            while state:
                tot = sum(st[1] for st in state)
                for st in state:
                    st[2] += st[1] / tot
                st = max(state, key=lambda s: s[2])
                st[2] -= 1.0
                if next(st[0], "__done__") == "__done__":
                    state.remove(st)

        # ---- schedule ----
        # A(hc) covers chunk hc//2; B(b) needs chunks <= b roped and v'd;
        # C(b) needs all of B(b).  Fillers keep exp off the PE critical
        # path.  Each B's head is pre-started inside the previous C
        # interleave (after the A epilogue that ropes its chunk) so its
        # counting-semaphore thresholds exclude that C's final evictions.
        b1 = gen_b(1)
        b2 = gen_b(2)
        b3 = gen_b(3, lookahead=True)
        run(gen_a_full(0))
        a2 = gen_a(2)
        run(take(a2, 6))     # cover chunk 0's rope latency (DVE) with A2
                             # matmuls before B0's rope-dependent scores join
        interleave((gen_b(0), 1), (a2, 2))
        # wo streams here (11us of wire), after the startup x/weight crunch
        # but a full interleave ahead of its first phase-C consumer.
        load_wo(0)
        load_wo(1)
        interleave((gen_c(0), 2), (chain(gen_a(3), take(b1, 9)), 3))
        interleave((b1, 2), (gen_a(4), 3))
        interleave((gen_c(1), 4), (chain(gen_a(5), take(b2, 13)), 7))
        interleave((b2, 1), (gen_a(6), 1))
        interleave((gen_c(2, 0, 6), 3), (chain(gen_a(7), take(b3, 20)), 8))
        interleave((b3, 6), (gen_c(2, 6, 8, sync_ok=True), 1))
        run(gen_c(3, tail=True))

    return nc


def _make_in_maps(x, freqs_cis, wqkv, wo):
    scale = np.float32(1.0 / np.sqrt(HD))
    xT = np.asarray(x)[0].T.astype(NPBF16)               # (DIM, S)
    # x sub-slabs (32, 128, 8, 256): slab hc*4+sub holds k-tiles sub*8..+7
    # (partition-major) for half-chunk hc -- each is one contiguous DMA.
    xkt = xT.reshape(32, 128, S)
    # chunk 0 (half-chunks 0,1) interleaved [h0s0, h1s0, h0s1, ...] for the
    # full-chunk A phase; chunks 1-3 sequential per half-chunk.
    order = [(hc, sub) for sub in range(4) for hc in (0, 1)] + \
            [(hc, sub) for hc in range(2, 8) for sub in range(4)]
    x5 = np.ascontiguousarray(np.stack(
        [xkt[sub * 8:(sub + 1) * 8, :, hc * 256:(hc + 1) * 256].transpose(1, 0, 2)
         for hc, sub in order]))
    NCH, CHUNK = 4, 512
    cos = freqs_cis[:, :, 0].T.astype(np.float32)        # (64, S)
    sin = freqs_cis[:, :, 1].T.astype(np.float32)
    cosT = np.concatenate([cos, cos], 0)                 # (128, S)
    sinT = np.concatenate([-sin, sin], 0)
    # (128, NCH, 2, CHUNK): per chunk one contiguous cos||sin slab
    csT = np.ascontiguousarray(
        np.stack([cosT.reshape(128, NCH, CHUNK),
                  sinT.reshape(128, NCH, CHUNK)], axis=2))
    ones = np.ones((128, 128), NPBF16)
    kp = np.arange(128)[:, None]
    qp = np.arange(128)[None, :]
    maskT = np.where(kp <= qp, 0.0, -1e30).astype(np.float32)

    in_maps = []
    for c in range(N_CORES):
        rows = [wqkv[128 * (NHL * c + h) + PERM] * scale for h in range(NHL)]
        rows.append(wqkv[NH * HD + 128 * c + PERM])
        wqkT = np.concatenate(rows, 0).T                  # (DIM, 640)
        wvT = wqkv[(NH + NKV) * HD + 128 * c:(NH + NKV) * HD + 128 * (c + 1)].T
        wqkvT = np.concatenate([wqkT, wvT], 1)            # (DIM, 768)
        # pre-swizzle (kt*128+p, m) -> (p, kt, m): device group loads become
        # contiguous slab reads instead of strided gathers
        wqkvT = np.ascontiguousarray(
            wqkvT.reshape(32, 128, -1).transpose(1, 0, 2)).astype(NPBF16)
        woT = wo[:, 128 * NHL * c:128 * NHL * (c + 1)].T  # (512, DIM)
        woT = np.ascontiguousarray(
            woT.reshape(NHL, 128, DIM).transpose(1, 0, 2)).astype(NPBF16)
        in_maps.append({
            "x5": x5, "wqkvT": wqkvT, "woT": woT,
            "csT": csT, "onesW": ones, "maskT": maskT,
        })
    return in_maps


def kernel(x, freqs_cis, wqkv, wo):
    x = np.asarray(x, dtype=np.float32)
    freqs_cis = np.asarray(freqs_cis, dtype=np.float32)
    wqkv = np.asarray(wqkv, dtype=np.float32)
    wo = np.asarray(wo, dtype=np.float32)

    in_maps = _make_in_maps(x, freqs_cis, wqkv, wo)
    nc = bacc.Bacc("TRN2", target_bir_lowering=False, debug=False,
                   num_devices=N_CORES)
    build_attention_kernel(nc, S=S, DIM=DIM)
    nc.compile()
    res = run_bass_kernel_spmd(nc, in_maps, core_ids=list(range(N_CORES)))

    acc = np.zeros((S, DIM), np.float32)
    for r in res.results:
        acc += np.asarray(r["out"]).astype(np.float32)
    return acc[None]
